# revision 20
# baseline (speedup 1.0000x reference)
"""Trainium2 Bass kernel for nn_ADAM_SINDy_MODEL (568-term SINDy library regression).

Math: the reference computes terms[B,T,568] @ a with a data-independent
column mask. Everything collapses per row to

    out = c0*con + w_lin.x + w_hill.g + x^T (U x + W_drug d + W_mm2^T g)

with g = x/(0.5+x) = 1 - r, r = 1/(2x+1) = exp(-ln(2x+1)).  With the
on-chip feature vector u = [con, x(21), d(5), 1, 0pad(4), junk, r(21)]
(54 rows, feature-major):

    H = W1^T u            (22 rows; H_0 = 0)
    out_n = w86 . [u ; pad ; u[0:22]*H]   (86-row stacked tile)

Per core (pure data parallelism over batch): 32768 rows in 16 chunks of
2048.  The host uploads candidates feature-major [32, ROWS] fp32 (same
bytes, layout prep only), so no on-chip transposes are needed at all:
DMA -> cast fp16 -> ln/exp (ACT LUT) -> mm1 (fp16) -> P-mult (DVE) ->
16 data-as-weights reduce matmuls (fp16, FWL) -> copy out -> DMA.
"""

import os
import sys

import numpy as np

if "/opt/trn_rl_repo" not in sys.path:
    sys.path.insert(0, "/opt/trn_rl_repo")

NX, ND = 21, 5
B, T = 128, 2048
NCORES = 8
BPC = B // NCORES          # batches per core
ROWS = BPC * T             # rows per core
CHUNK = 2048
NCHUNK = ROWS // CHUNK     # 16
NSUB = CHUNK // 512        # mm1 sub-slices
NBLK = CHUNK // 128        # reduce-matmul blocks per chunk
NFEAT_IN = 32              # dma'd rows: con,x,d,1,0,0,0,0
RBASE = 32                 # r block: junk at 32, r at 33..53
FEATK = 54                 # mm1 contraction rows
PBASE = 64                 # P block base partition
TALL = PBASE + 22

_CACHE = {}


def _build_coeffs(a, lin_idx, drug_idx, bilin_idx, mm2_idx, hill_idx, uses_self):
    a = np.asarray(a, np.float64).reshape(-1)
    uses_self = np.asarray(uses_self).astype(bool).reshape(-1)
    lin_idx = np.asarray(lin_idx).reshape(-1)
    drug_idx = np.asarray(drug_idx).reshape(-1, 2)
    bilin_idx = np.asarray(bilin_idx).reshape(-1, 2)
    mm2_idx = np.asarray(mm2_idx).reshape(-1, 2)
    hill_idx = np.asarray(hill_idx).reshape(-1)

    n = a.shape[0]
    idx = np.arange(n)
    zero = np.where(uses_self, a > 0.0, a < 0.0) & (idx >= 2)
    ae = np.where(zero, 0.0, a)

    nl, ndg, nb, nm, nh = (
        len(lin_idx), len(drug_idx), len(bilin_idx), len(mm2_idx), len(hill_idx),
    )
    o2 = 1 + nl
    o3 = o2 + ndg
    o4 = o3 + nb
    o5 = o4 + nm

    c0 = ae[0]
    w_lin = np.zeros(NX)
    np.add.at(w_lin, lin_idx, ae[1:o2])
    W_drug = np.zeros((NX, ND))
    np.add.at(W_drug, (drug_idx[:, 0], drug_idx[:, 1]), ae[o2:o3])
    U = np.zeros((NX, NX))
    np.add.at(U, (bilin_idx[:, 0], bilin_idx[:, 1]), ae[o3:o4])
    W_mm2 = np.zeros((NX, NX))
    np.add.at(W_mm2, (mm2_idx[:, 0], mm2_idx[:, 1]), ae[o4:o5])
    w_hill = np.zeros(NX)
    np.add.at(w_hill, hill_idx, ae[o5 : o5 + nh])

    # u rows: 0=con, 1..21=x, 22..26=d, 27=1, 28..31=0, 32=junk, 33..53=r
    W1 = np.zeros((FEATK, 22))
    W1[1 : 1 + NX, 1:] = U.T
    W1[22 : 22 + ND, 1:] = W_drug.T
    W1[33 : 33 + NX, 1:] = -W_mm2

    v = np.zeros(FEATK)
    v[0] = c0
    v[1 : 1 + NX] = w_lin + W_mm2.sum(axis=0)
    v[27] = w_hill.sum()
    v[33 : 33 + NX] = -w_hill

    w86 = np.zeros(TALL)
    w86[:FEATK] = v
    w86[PBASE:] = 1.0
    return W1.astype(np.float16), w86.astype(np.float16).reshape(TALL, 1)


def _build_nc():
    import concourse.bacc as bacc
    import concourse.tile as tile
    from concourse import mybir

    f32 = mybir.dt.float32
    f16 = mybir.dt.float16
    Act = mybir.ActivationFunctionType

    import concourse.hw_specs as hw_specs

    if not getattr(hw_specs, "_act_tables_pinned", False):
        _real_get = hw_specs.get_activation_tables

        def _pinned(arch):
            t = dict(_real_get(arch))
            return {
                k: (v if k == "natural_log_exp_and_others" else set())
                for k, v in t.items()
            }

        hw_specs.get_activation_tables = _pinned
        bacc.get_activation_tables = _pinned
        hw_specs._act_tables_pinned = True

    nc = bacc.Bacc(
        "TRN2", target_bir_lowering=False, debug=False, num_devices=NCORES
    )
    candT = nc.declare_dram_parameter("candT", [NFEAT_IN, ROWS], f32, isOutput=False)
    w1_d = nc.declare_dram_parameter("w1", [FEATK, 22], f16, isOutput=False)
    w86_d = nc.declare_dram_parameter("w86", [TALL, 1], f16, isOutput=False)
    out_d = nc.declare_dram_parameter("out", [ROWS], f32, isOutput=True)

    with tile.TileContext(nc) as tc:
        with (
            tc.tile_pool(name="const", bufs=1) as cpool,
            tc.tile_pool(name="sb32", bufs=3) as spool,
            tc.tile_pool(name="tall", bufs=3) as tpool,
            tc.tile_pool(name="osb", bufs=4) as opool,
            tc.tile_pool(name="psH", bufs=2, space="PSUM") as psH,
            tc.tile_pool(name="psO", bufs=2, space="PSUM") as psO,
        ):
            w1_sb = cpool.tile([FEATK, 22], f16)
            nc.sync.dma_start(out=w1_sb[:], in_=w1_d[:, :])
            w86_sb = cpool.tile([TALL, 1], f16)
            nc.sync.dma_start(out=w86_sb[:], in_=w86_d[:, :])

            for c in range(NCHUNK):
                r0 = c * CHUNK
                sb32 = spool.tile([NFEAT_IN, CHUNK], f32)
                nc.sync.dma_start(
                    out=sb32[0:16, :], in_=candT[0:16, r0 : r0 + CHUNK]
                )
                nc.sync.dma_start(
                    out=sb32[16:NFEAT_IN, :], in_=candT[16:NFEAT_IN, r0 : r0 + CHUNK]
                )

                tall = tpool.tile([TALL, CHUNK], f16)
                # cast con/x/d/1/pads to fp16
                nc.vector.tensor_copy(out=tall[0:NFEAT_IN, :], in_=sb32[:])
                # zero rows 32..63 (gap rows stay 0; ln/exp overwrite 32..53)
                nc.gpsimd.memset(tall[RBASE : PBASE, :], 0.0)
                # r = exp(-ln(2x+1)); base-0 22-row ops (row 32 is junk 1/(2con+1))
                nc.scalar.activation(
                    out=tall[RBASE : RBASE + 22, :],
                    in_=tall[0:22, :],
                    func=Act.Ln,
                    bias=1.0,
                    scale=2.0,
                )
                nc.scalar.activation(
                    out=tall[RBASE : RBASE + 22, :],
                    in_=tall[RBASE : RBASE + 22, :],
                    func=Act.Exp,
                    bias=0.0,
                    scale=-1.0,
                )

                for h in range(2):
                    H = psH.tile([22, CHUNK // 2], f32)
                    for j in range(2):
                        s = h * (CHUNK // 2) + j * 512
                        nc.tensor.matmul(
                            out=H[:, j * 512 : (j + 1) * 512],
                            lhsT=w1_sb[:],
                            rhs=tall[0:FEATK, s : s + 512],
                            start=True,
                            stop=True,
                            skip_group_check=True,
                        )
                    hs = h * (CHUNK // 2)
                    nc.vector.tensor_mul(
                        out=tall[PBASE : PBASE + 22, hs : hs + CHUNK // 2],
                        in0=tall[0:22, hs : hs + CHUNK // 2],
                        in1=H[:],
                    )

                o128 = psO.tile([128, NBLK], f32)
                for j in range(NBLK):
                    nc.tensor.matmul(
                        out=o128[:, j : j + 1],
                        lhsT=tall[:, j * 128 : (j + 1) * 128],
                        rhs=w86_sb[:],
                        start=True,
                        stop=True,
                        skip_group_check=True,
                    )

                osb = opool.tile([128, NBLK], f32)
                nc.scalar.activation(out=osb[:], in_=o128[:], func=Act.Copy)
                dst = out_d[r0 : r0 + CHUNK].rearrange("(q p) -> p q", p=128)
                nc.sync.dma_start(out=dst, in_=osb[:])

    nc.compile()
    return nc


def _get_nc():
    if "nc" not in _CACHE:
        _CACHE["nc"] = _build_nc()
    return _CACHE["nc"]


def _ensure_ntff_hook():
    """The agent image's antenv lacks axon_hooks; synthesize it from the
    boot module's ctypes NTFF driver so trace=True can capture profiles."""
    try:
        from antenv.axon_hooks import get_axon_ntff_profile_hook  # noqa: F401

        return
    except ImportError:
        pass
    try:
        import types

        import antenv
        from trn_agent_boot.trn_boot import _ntff_profile_via_ctypes

        hook = _ntff_profile_via_ctypes("/opt/axon/libaxon_pjrt.so")
        mod = types.ModuleType("antenv.axon_hooks")
        holder = {"hook": hook}
        mod.get_axon_ntff_profile_hook = lambda: holder["hook"]
        mod.set_axon_ntff_profile_hook = lambda h: holder.update(hook=h)
        sys.modules["antenv.axon_hooks"] = mod
        antenv.axon_hooks = mod
    except Exception as e:  # degrade to untraced
        print(f"ntff hook setup failed: {e}", file=sys.stderr)


def kernel(**inputs) -> np.ndarray:
    from concourse.bass_utils import run_bass_kernel_spmd

    cand = np.ascontiguousarray(np.asarray(inputs["candidates"], dtype=np.float32))
    assert cand.shape == (B, T, 27), cand.shape
    W1, w86 = _build_coeffs(
        inputs["a"],
        inputs["lin_idx"],
        inputs["drug_idx"],
        inputs["bilin_idx"],
        inputs["mm2_idx"],
        inputs["hill_idx"],
        inputs["uses_self"],
    )

    nc = _get_nc()
    in_maps = []
    for i in range(NCORES):
        shard = cand[i * BPC : (i + 1) * BPC].reshape(ROWS, 27)
        ct = np.zeros((NFEAT_IN, ROWS), np.float32)
        ct[0:27] = shard.T
        ct[27] = 1.0
        in_maps.append({"candT": np.ascontiguousarray(ct), "w1": W1, "w86": w86})

    trace = os.environ.get("BASS_TRACE", "") == "1"
    if trace:
        _ensure_ntff_hook()
    res = run_bass_kernel_spmd(
        nc, in_maps, core_ids=list(range(NCORES)), trace=trace
    )
    if res.exec_time_ns is not None:
        print(f"HW exec time: {res.exec_time_ns} ns")
        _CACHE["exec_time_ns"] = res.exec_time_ns

    out = np.concatenate(
        [res.results[i]["out"].reshape(BPC, T) for i in range(NCORES)], axis=0
    )
    return out.astype(np.float32)


# revision 21
# speedup vs baseline: 1.1902x; 1.1902x over previous
"""Trainium2 Bass kernel for nn_ADAM_SINDy_MODEL (568-term SINDy library regression).

Math: the reference computes terms[B,T,568] @ a with a data-independent
column mask. Everything collapses per row to

    out = c0*con + w_lin.x + w_hill.g + x^T (U x + W_drug d + W_mm2^T g)

with g = x/(0.5+x) = 1 - r, r = 1/(2x+1) = exp(-ln(2x+1)).  With the
on-chip feature vector u = [con, x(21), d(5), 1, 0pad(4), junk, r(21)]
(54 rows, feature-major):

    H = W1^T u            (22 rows; H_0 = 0)
    out_n = w86 . [u ; pad ; u[0:22]*H]   (86-row stacked tile)

Per core (pure data parallelism over batch): 32768 rows in 16 chunks of
2048.  The host uploads candidates feature-major [32, ROWS] fp32 (same
bytes, layout prep only), so no on-chip transposes are needed at all:
DMA -> cast fp16 -> ln/exp (ACT LUT) -> mm1 (fp16) -> P-mult (DVE) ->
16 data-as-weights reduce matmuls (fp16, FWL) -> copy out -> DMA.
"""

import os
import sys

import numpy as np

if "/opt/trn_rl_repo" not in sys.path:
    sys.path.insert(0, "/opt/trn_rl_repo")

NX, ND = 21, 5
B, T = 128, 2048
NCORES = 8
BPC = B // NCORES          # batches per core
ROWS = BPC * T             # rows per core
CHUNK = 2048
NCHUNK = ROWS // CHUNK     # 16
NSUB = CHUNK // 512        # mm1 sub-slices
NBLK = CHUNK // 128        # reduce-matmul blocks per chunk
NFEAT_IN = 32              # dma'd rows: con,x,d,1,0,0,0,0
RBASE = 32                 # r block: junk at 32, r at 33..53
FEATK = 54                 # mm1 contraction rows
PBASE = 64                 # P block base partition
TALL = PBASE + 22

_CACHE = {}


def _build_coeffs(a, lin_idx, drug_idx, bilin_idx, mm2_idx, hill_idx, uses_self):
    a = np.asarray(a, np.float64).reshape(-1)
    uses_self = np.asarray(uses_self).astype(bool).reshape(-1)
    lin_idx = np.asarray(lin_idx).reshape(-1)
    drug_idx = np.asarray(drug_idx).reshape(-1, 2)
    bilin_idx = np.asarray(bilin_idx).reshape(-1, 2)
    mm2_idx = np.asarray(mm2_idx).reshape(-1, 2)
    hill_idx = np.asarray(hill_idx).reshape(-1)

    n = a.shape[0]
    idx = np.arange(n)
    zero = np.where(uses_self, a > 0.0, a < 0.0) & (idx >= 2)
    ae = np.where(zero, 0.0, a)

    nl, ndg, nb, nm, nh = (
        len(lin_idx), len(drug_idx), len(bilin_idx), len(mm2_idx), len(hill_idx),
    )
    o2 = 1 + nl
    o3 = o2 + ndg
    o4 = o3 + nb
    o5 = o4 + nm

    c0 = ae[0]
    w_lin = np.zeros(NX)
    np.add.at(w_lin, lin_idx, ae[1:o2])
    W_drug = np.zeros((NX, ND))
    np.add.at(W_drug, (drug_idx[:, 0], drug_idx[:, 1]), ae[o2:o3])
    U = np.zeros((NX, NX))
    np.add.at(U, (bilin_idx[:, 0], bilin_idx[:, 1]), ae[o3:o4])
    W_mm2 = np.zeros((NX, NX))
    np.add.at(W_mm2, (mm2_idx[:, 0], mm2_idx[:, 1]), ae[o4:o5])
    w_hill = np.zeros(NX)
    np.add.at(w_hill, hill_idx, ae[o5 : o5 + nh])

    # u rows: 0=con, 1..21=x, 22..26=d, 27=1, 28..31=0, 32=junk, 33..53=r
    W1 = np.zeros((FEATK, 22))
    W1[1 : 1 + NX, 1:] = U.T
    W1[22 : 22 + ND, 1:] = W_drug.T
    W1[33 : 33 + NX, 1:] = -W_mm2

    v = np.zeros(FEATK)
    v[0] = c0
    v[1 : 1 + NX] = w_lin + W_mm2.sum(axis=0)
    v[27] = w_hill.sum()
    v[33 : 33 + NX] = -w_hill

    w86 = np.zeros(TALL)
    w86[:FEATK] = v
    w86[PBASE:] = 1.0
    return W1.astype(np.float16), w86.astype(np.float16).reshape(TALL, 1)


def _build_nc():
    import concourse.bacc as bacc
    import concourse.tile as tile
    from concourse import mybir

    f32 = mybir.dt.float32
    f16 = mybir.dt.float16
    Act = mybir.ActivationFunctionType

    import concourse.hw_specs as hw_specs

    if not getattr(hw_specs, "_act_tables_pinned", False):
        _real_get = hw_specs.get_activation_tables

        def _pinned(arch):
            t = dict(_real_get(arch))
            return {
                k: (v if k == "natural_log_exp_and_others" else set())
                for k, v in t.items()
            }

        hw_specs.get_activation_tables = _pinned
        bacc.get_activation_tables = _pinned
        hw_specs._act_tables_pinned = True

    nc = bacc.Bacc(
        "TRN2", target_bir_lowering=False, debug=False, num_devices=NCORES
    )
    candT = nc.declare_dram_parameter("candT", [NFEAT_IN, ROWS], f32, isOutput=False)
    w1_d = nc.declare_dram_parameter("w1", [FEATK, 22], f16, isOutput=False)
    w86_d = nc.declare_dram_parameter("w86", [TALL, 1], f16, isOutput=False)
    out_d = nc.declare_dram_parameter("out", [ROWS], f32, isOutput=True)

    with tile.TileContext(nc) as tc:
        with (
            tc.tile_pool(name="const", bufs=1) as cpool,
            tc.tile_pool(name="sb32", bufs=4) as spool,
            tc.tile_pool(name="tall", bufs=4) as tpool,
            tc.tile_pool(name="osb", bufs=4) as opool,
            tc.tile_pool(name="psH", bufs=3, space="PSUM") as psH,
            tc.tile_pool(name="psO", bufs=2, space="PSUM") as psO,
        ):
            w1_sb = cpool.tile([FEATK, 22], f16)
            nc.sync.dma_start(out=w1_sb[:], in_=w1_d[:, :])
            w86_sb = cpool.tile([TALL, 1], f16)
            nc.sync.dma_start(out=w86_sb[:], in_=w86_d[:, :])

            for c in range(NCHUNK):
                r0 = c * CHUNK
                sb32 = spool.tile([NFEAT_IN, CHUNK], f32)
                nc.sync.dma_start(
                    out=sb32[0:16, :], in_=candT[0:16, r0 : r0 + CHUNK]
                )
                nc.sync.dma_start(
                    out=sb32[16:NFEAT_IN, :], in_=candT[16:NFEAT_IN, r0 : r0 + CHUNK]
                )

                tall = tpool.tile([TALL, CHUNK], f16)
                # cast con/x/d/1/pads to fp16
                nc.vector.tensor_copy(out=tall[0:NFEAT_IN, :], in_=sb32[:])
                # r = exp(-ln(2x+1)) on all 32 rows: rows 32..63 get finite
                # junk (1/(2v+1), =1.0 for the zero pads); reduce weights
                # there are zero, so no memset is needed at all
                nc.scalar.activation(
                    out=tall[RBASE : RBASE + 32, :],
                    in_=tall[0:32, :],
                    func=Act.Ln,
                    bias=1.0,
                    scale=2.0,
                )
                nc.scalar.activation(
                    out=tall[RBASE : RBASE + 32, :],
                    in_=tall[RBASE : RBASE + 32, :],
                    func=Act.Exp,
                    bias=0.0,
                    scale=-1.0,
                )

                for h in range(2):
                    H = psH.tile([22, CHUNK // 2], f32)
                    for j in range(2):
                        s = h * (CHUNK // 2) + j * 512
                        nc.tensor.matmul(
                            out=H[:, j * 512 : (j + 1) * 512],
                            lhsT=w1_sb[:],
                            rhs=tall[0:FEATK, s : s + 512],
                            start=True,
                            stop=True,
                            skip_group_check=True,
                        )
                    hs = h * (CHUNK // 2)
                    nc.vector.tensor_mul(
                        out=tall[PBASE : PBASE + 22, hs : hs + CHUNK // 2],
                        in0=tall[0:22, hs : hs + CHUNK // 2],
                        in1=H[:],
                    )

                o128 = psO.tile([128, NBLK], f32)
                for j in range(NBLK):
                    nc.tensor.matmul(
                        out=o128[:, j : j + 1],
                        lhsT=tall[:, j * 128 : (j + 1) * 128],
                        rhs=w86_sb[:],
                        start=True,
                        stop=True,
                        skip_group_check=True,
                    )

                osb = opool.tile([128, NBLK], f32)
                nc.vector.tensor_copy(out=osb[:], in_=o128[:])
                dst = out_d[r0 : r0 + CHUNK].rearrange("(q p) -> p q", p=128)
                nc.sync.dma_start(out=dst, in_=osb[:])

    nc.compile()
    return nc


def _get_nc():
    if "nc" not in _CACHE:
        _CACHE["nc"] = _build_nc()
    return _CACHE["nc"]


def _ensure_ntff_hook():
    """The agent image's antenv lacks axon_hooks; synthesize it from the
    boot module's ctypes NTFF driver so trace=True can capture profiles."""
    try:
        from antenv.axon_hooks import get_axon_ntff_profile_hook  # noqa: F401

        return
    except ImportError:
        pass
    try:
        import types

        import antenv
        from trn_agent_boot.trn_boot import _ntff_profile_via_ctypes

        hook = _ntff_profile_via_ctypes("/opt/axon/libaxon_pjrt.so")
        mod = types.ModuleType("antenv.axon_hooks")
        holder = {"hook": hook}
        mod.get_axon_ntff_profile_hook = lambda: holder["hook"]
        mod.set_axon_ntff_profile_hook = lambda h: holder.update(hook=h)
        sys.modules["antenv.axon_hooks"] = mod
        antenv.axon_hooks = mod
    except Exception as e:  # degrade to untraced
        print(f"ntff hook setup failed: {e}", file=sys.stderr)


def kernel(**inputs) -> np.ndarray:
    from concourse.bass_utils import run_bass_kernel_spmd

    cand = np.ascontiguousarray(np.asarray(inputs["candidates"], dtype=np.float32))
    assert cand.shape == (B, T, 27), cand.shape
    W1, w86 = _build_coeffs(
        inputs["a"],
        inputs["lin_idx"],
        inputs["drug_idx"],
        inputs["bilin_idx"],
        inputs["mm2_idx"],
        inputs["hill_idx"],
        inputs["uses_self"],
    )

    nc = _get_nc()
    in_maps = []
    for i in range(NCORES):
        shard = cand[i * BPC : (i + 1) * BPC].reshape(ROWS, 27)
        ct = np.zeros((NFEAT_IN, ROWS), np.float32)
        ct[0:27] = shard.T
        ct[27] = 1.0
        in_maps.append({"candT": np.ascontiguousarray(ct), "w1": W1, "w86": w86})

    trace = os.environ.get("BASS_TRACE", "") == "1"
    if trace:
        _ensure_ntff_hook()
    res = run_bass_kernel_spmd(
        nc, in_maps, core_ids=list(range(NCORES)), trace=trace
    )
    if res.exec_time_ns is not None:
        print(f"HW exec time: {res.exec_time_ns} ns")
        _CACHE["exec_time_ns"] = res.exec_time_ns

    out = np.concatenate(
        [res.results[i]["out"].reshape(BPC, T) for i in range(NCORES)], axis=0
    )
    return out.astype(np.float32)
